# revision 31
# baseline (speedup 1.0000x reference)
"""Trainium2 Bass kernel for nn_MemoryAttention (causal single-head attention
with SiLU-gated output projection), sequence-parallel across 8 NeuronCores.

v5 strategy (per core c):
  - q rows owned: 4 slots of 256 rows: tile t = c + 8*s (strided assignment
    balances causal work; every core runs an identical instruction stream).
  - fp8e4 (DoubleRow) for Q/K projections, QK^T logits, and all of slots
    1-3's PV via the delta decomposition P = 1 + delta with multiplicative
    masks folded in before the -1 (masked keys give delta = -1 exactly,
    cancelled by hi+lo bf16 colsum(V8) seeds computed on the producer).
  - Slot 0 (fully boundary, small rowsums) keeps the bf16 masked path over
    bf16 V shipped only for level 0.
  - Logits are computed ONCE per key-tile for every slot that sees it, at
    free dim 512 (2 q-slots per matmul), during the EARLIEST slot's pass;
    later slots' deltas are forwarded through fp8 SBUF stores (~24KB/part).
  - Two collectives only: G0 (slot-0 payload: kt|v8|v16|colsums) posted
    ~20us in, and G123 (slots 1-3: kt|v8|colsum8 hi+lo) posted right after
    the last V projection.  Collectives here cost ~35us nearly independent
    of size, so fewer is faster.
  - Slot epilogue: H / rowsums, SiLU, DMA-transpose (XBAR) of G, out proj.
"""

import numpy as np
import ml_dtypes

import concourse.bass as bass
import concourse.tile as tile
from concourse import bacc, mybir
from concourse.bass_utils import run_bass_kernel_spmd
from concourse.masks import make_identity

P = 128
D = 1024
SEQ = 8192
NCORES = 8
NSLOTS = 4
WSCALE = 64.0
EXP_SCALE = 0.03125 / (WSCALE * WSCALE)

# slot-0 payload rows (units of [., 1024] bf16): kt fp8 | v8 fp8 | v bf16 | cs
R0_KT = 0
R0_V8 = P
R0_V16 = 2 * P
R0_CS = 4 * P
R0_TOT = 4 * P + 3
# slots 1-3 payload rows: kt fp8 | v8 fp8 | colsum(v8) bf16 hi+lo
R_KT = 0
R_V8 = P
R_CS8 = 2 * P
R_TOT = 2 * P + 2

F32 = mybir.dt.float32
BF16 = mybir.dt.bfloat16
FP8 = mybir.dt.float8e4
AF = mybir.ActivationFunctionType
DR = mybir.MatmulPerfMode.DoubleRow
AX = mybir.AxisListType
ALU = mybir.AluOpType


def build_kernel():
    nc = bacc.Bacc(None, target_bir_lowering=False, num_devices=NCORES)

    xb_ext = nc.declare_dram_parameter("xb", [D, D], BF16, isOutput=False)
    x8_ext = nc.declare_dram_parameter("x8", [D, D], FP8, isOutput=False)
    wq_ext = nc.declare_dram_parameter("wq", [D, D], FP8, isOutput=False)
    wk_ext = nc.declare_dram_parameter("wk", [D, D], FP8, isOutput=False)
    wv1_ext = nc.declare_dram_parameter("wv1", [D, D], BF16, isOutput=False)
    wv2_ext = nc.declare_dram_parameter("wv2", [D, D], BF16, isOutput=False)
    mask_ext = nc.declare_dram_parameter("masks", [64, P, 256], BF16, isOutput=False)
    o_ext = nc.declare_dram_parameter("o", [NSLOTS, 2, P, D], F32, isOutput=True)

    kv0_local = nc.dram_tensor("kv0_local", [R0_TOT, D], BF16)
    kv0_gath = nc.dram_tensor("kv0_gath", [NCORES, R0_TOT, D], BF16, addr_space="Shared")
    kv_local = nc.dram_tensor("kv_local", [3, R_TOT, D], BF16)
    kvB_gath = nc.dram_tensor(
        "kvB_gath", [3, NCORES, R_TOT, D], BF16, addr_space="Shared"
    )

    def wload(nc, pool, ext, tag, dt):
        t = pool.tile([P, 8, D], dt, tag=tag, name=tag)
        nc.sync.dma_start(out=t, in_=ext[:].rearrange("(sub p) s -> p sub s", p=P))
        return t

    with tile.TileContext(nc) as tc:
        singles_ctx = tc.tile_pool(name="singles", bufs=1)
        singles = singles_ctx.__enter__()

        ones_sb = singles.tile([P, 1], BF16)
        nc.vector.memset(ones_sb, 1.0)
        ones_row = singles.tile([1, P], BF16)
        nc.vector.memset(ones_row, 1.0)
        ones8_2 = singles.tile([P, 2, 1], FP8)
        nc.vector.memset(ones8_2, 1.0)
        ones8_1 = singles.tile([P, 1], FP8)
        nc.vector.memset(ones8_1, 1.0)
        ones8w = singles.tile([8, P], BF16)
        nc.vector.memset(ones8w, 1.0)
        cnt_sb = singles.tile([1, 2 * NSLOTS], BF16)
        for s in range(NSLOTS):
            cval = 0.0 if s == 0 else float(2048 * (s + 1))
            nc.vector.memset(cnt_sb[:, 2 * s : 2 * s + 2], cval)
        ident_sb = singles.tile([P, P], BF16)
        make_identity(nc, ident_sb)
        qt_sb = singles.tile([P, 8, D], FP8)

        with (
            tc.tile_pool(name="projw", bufs=1) as projw,
            tc.tile_pool(name="projout", bufs=4) as projout,
            tc.tile_pool(name="ppsum", bufs=1, space="PSUM") as ppsum,
            tc.tile_pool(name="vpsum", bufs=4, space="PSUM") as vpsum,
            tc.tile_pool(name="cpsum", bufs=1, space="PSUM") as cpsum,
        ):
            def pairload(pool, ext, tag, dt):
                t = pool.tile([P, 8, D], dt, tag=tag, name=tag)
                v = ext[:].rearrange("(sub p) s -> p sub s", p=P)
                for p4 in range(4):
                    nc.sync.dma_start(
                        out=t[:, 2 * p4 : 2 * p4 + 2, :], in_=v[:, 2 * p4 : 2 * p4 + 2, :]
                    )
                return t

            x8_v = x8_ext[:].rearrange("(sub p) s -> p sub s", p=P)
            wk_v = wk_ext[:].rearrange("(sub p) s -> p sub s", p=P)
            x8_sb = projw.tile([P, 8, D], FP8, tag="x8", name="x8")
            wk8 = projw.tile([P, 8, D], FP8, tag="wk", name="wk")
            for p4 in range(4):
                pr = slice(2 * p4, 2 * p4 + 2)
                nc.sync.dma_start(out=wk8[:, pr, :], in_=wk_v[:, pr, :])
                nc.sync.dma_start(out=x8_sb[:, pr, :], in_=x8_v[:, pr, :])
            xb_sb = pairload(projw, xb_ext, "xb", BF16)
            wv1_sb = pairload(projw, wv1_ext, "wv1", BF16)
            wq8 = pairload(projw, wq_ext, "wq", FP8)

            def kqt_proj(w8, cols, out_cb):
                for m in range(8):
                    acc = ppsum.tile([P, 512], F32, tag=f"proj{m % 2}", name=f"kq{m}")
                    for p4 in range(4):
                        nc.tensor.matmul(
                            acc,
                            lhsT=w8[:, 2 * p4 : 2 * p4 + 2, m * P : (m + 1) * P],
                            rhs=x8_sb[:, 2 * p4 : 2 * p4 + 2, cols],
                            start=(p4 == 0),
                            stop=(p4 == 3),
                            perf_mode=DR,
                        )
                    out_cb(m, acc)

            def gather(in_ap, out_ap):
                nc.gpsimd.collective_compute(
                    "AllGather",
                    mybir.AluOpType.bypass,
                    replica_groups=[list(range(NCORES))],
                    ins=[in_ap],
                    outs=[out_ap],
                )

            def kproj_pair(p01):
                cols = slice(p01 * 512, (p01 + 1) * 512)
                kt_out = projout.tile([P, 8, 512], FP8, tag="kt_out", name="kto")
                kqt_proj(
                    wk8,
                    cols,
                    lambda m, acc: nc.vector.tensor_copy(out=kt_out[:, m, :], in_=acc),
                )
                for gg in range(2):
                    g = 2 * p01 + gg
                    dst = (
                        kv0_local[R0_KT : R0_KT + P]
                        if g == 0
                        else kv_local[g - 1, R_KT : R_KT + P]
                    )
                    nc.sync.dma_start(
                        out=dst.bitcast(FP8).rearrange("p (m c) -> p m c", m=8),
                        in_=kt_out[:, :, gg * 256 : (gg + 1) * 256],
                    )

            def vproj(g):
                v8_out = projout.tile([P, 2, D], FP8, tag="v8_out", name="v8o")
                for blk in range(2):
                    accs = [
                        vpsum.tile([P, 512], F32, tag="vproj", name=f"vp{h2}")
                        for h2 in range(2)
                    ]
                    bc = slice(g * 256 + blk * P, g * 256 + (blk + 1) * P)
                    for sub in range(8):
                        for h2 in range(2):
                            nc.tensor.matmul(
                                accs[h2],
                                lhsT=xb_sb[:, sub, bc],
                                rhs=wv1_sb[:, sub, h2 * 512 : (h2 + 1) * 512],
                                start=(sub == 0),
                                stop=(sub == 7),
                            )
                    with nc.allow_low_precision(reason="fp8 V feeds the delta path"):
                        for h2 in range(2):
                            nc.scalar.activation(
                                out=v8_out[:, blk, h2 * 512 : (h2 + 1) * 512],
                                in_=accs[h2],
                                func=AF.Copy,
                            )
                    if g == 0:
                        v_out = projout.tile([P, D], BF16, tag="v_out", name="vo")
                        for h2 in range(2):
                            nc.vector.tensor_copy(
                                out=v_out[:, h2 * 512 : (h2 + 1) * 512], in_=accs[h2]
                            )
                        nc.sync.dma_start(
                            out=kv0_local[R0_V16 + blk * P : R0_V16 + (blk + 1) * P],
                            in_=v_out,
                        )
                v8_dst = (
                    kv0_local[R0_V8 : R0_V8 + P]
                    if g == 0
                    else kv_local[g - 1, R_V8 : R_V8 + P]
                )
                nc.sync.dma_start(
                    out=v8_dst.bitcast(FP8), in_=v8_out.rearrange("p b d -> p (b d)")
                )
                if g > 0:
                    # colsum of the shipped v8 (fp32 psum), hi+lo bf16 split
                    # so it cancels the masked-key deltas to ~1e-4
                    cs = cpsum.tile([1, D], F32, tag="cs", name="cs8")
                    for dh in range(2):
                        for b in range(2):
                            nc.tensor.matmul(
                                cs[:, dh * 512 : (dh + 1) * 512],
                                lhsT=ones8_1,
                                rhs=v8_out[:, b, dh * 512 : (dh + 1) * 512],
                                start=(b == 0),
                                stop=(b == 1),
                            )
                    csh = projout.tile([1, D], BF16, tag="cs8h", name="cs8h")
                    csl = projout.tile([1, D], BF16, tag="cs8l", name="cs8l")
                    with nc.allow_low_precision(reason="bf16 hi+lo split of fp32 cs8"):
                        nc.vector.tensor_copy(out=csh, in_=cs)
                        nc.vector.tensor_sub(out=csl, in0=cs, in1=csh)
                    nc.sync.dma_start(out=kv_local[g - 1, R_CS8 : R_CS8 + 1], in_=csh)
                    nc.sync.dma_start(
                        out=kv_local[g - 1, R_CS8 + 1 : R_CS8 + 2], in_=csl
                    )

            def ship_exact_colsums():
                xsumL = projw.tile([P, 8, 3], BF16, tag="xsumL", name="xsumL")
                with nc.allow_low_precision(reason="bf16 colsum feeds hi+lo split"):
                    for lvl in range(3):
                        nc.vector.tensor_reduce(
                            out=xsumL[:, :, lvl : lvl + 1],
                            in_=xb_sb[:, :, lvl * 256 : (lvl + 1) * 256],
                            axis=AX.X,
                            op=ALU.add,
                        )
                for lvl in range(3):
                    cs = cpsum.tile([1, D], F32, tag="cs", name="cs")
                    for sub in range(8):
                        for dh in range(2):
                            nc.tensor.matmul(
                                cs[:, dh * 512 : (dh + 1) * 512],
                                lhsT=xsumL[:, sub, lvl : lvl + 1],
                                rhs=wv1_sb[:, sub, dh * 512 : (dh + 1) * 512],
                                start=(sub == 0),
                                stop=(sub == 7),
                            )
                    csb = projout.tile([1, D], BF16, tag="csb", name="csb")
                    nc.vector.tensor_copy(out=csb, in_=cs)
                    nc.sync.dma_start(
                        out=kv0_local[R0_CS + lvl : R0_CS + lvl + 1], in_=csb
                    )

            def qproj_pair(p01):
                cols = slice(p01 * 512, (p01 + 1) * 512)
                kqt_proj(
                    wq8,
                    cols,
                    lambda m, acc: nc.vector.tensor_copy(
                        out=qt_sb[:, m, p01 * 512 : (p01 + 1) * 512], in_=acc
                    ),
                )

            kproj_pair(0)
            ship_exact_colsums()
            vproj(0)
            gather(kv0_local[:], kv0_gath[:])
            vproj(1)
            gather(kv_local[0], kvB_gath[0])
            kproj_pair(1)
            vproj(2)
            gather(kv_local[1], kvB_gath[1])
            vproj(3)
            gather(kv_local[2], kvB_gath[2])
            qproj_pair(0)
            qproj_pair(1)

        # ---- attention ----------------------------------------------------
        with (
            tc.tile_pool(name="asingles", bufs=1) as asingles,
            tc.tile_pool(name="vpool", bufs=4) as vpool,
            tc.tile_pool(name="mpool", bufs=3) as mpool,
            tc.tile_pool(name="ptpool", bufs=3) as ptpool,
            tc.tile_pool(name="epool", bufs=2) as epool,
            tc.tile_pool(name="gpool", bufs=1) as gpool,
            tc.tile_pool(name="ltpsum", bufs=3, space="PSUM") as ltpsum,
            tc.tile_pool(name="hpsum", bufs=1, space="PSUM") as hpsum,
            tc.tile_pool(name="spsum", bufs=1, space="PSUM") as spsum,
        ):
            wv2_sb = wload(nc, asingles, wv2_ext, "wv2", BF16)
            v80_sb = asingles.tile([P, 8, 2048], FP8)
            # forward stores, written by the earliest pass that visits a
            # tile, read by later passes.  Passes run in order 1,2,3,0 so
            # every gather-gated load only blocks work needed after it.
            pt0_sb = asingles.tile([P, 8, 2, 256], BF16)   # slot0 masked P
            pt8s0 = asingles.tile([P, 8, 2, 512], FP8)     # tiles 0-7, slots 2,3
            pt8s1 = asingles.tile([P, 8, 2, 512], FP8)     # tiles 8-15, slots 2,3
            pt8s2 = asingles.tile([P, 8, 2, 256], FP8)     # tiles 16-23, slot 3

            corrf = asingles.tile([1, NSLOTS, D], F32)
            corrh = asingles.tile([1, NSLOTS, D], BF16)
            corrl = asingles.tile([1, NSLOTS, D], BF16)
            nc.vector.memset(corrf[:, 0], 0.0)
            nc.vector.memset(corrh[:, 0], 0.0)
            nc.vector.memset(corrl[:, 0], 0.0)

            def corr_level(g):
                csg = epool.tile([8, D], BF16, tag="csg", name=f"csg{g}")
                nc.sync.dma_start(out=csg, in_=kv0_gath[:, R0_CS + g])
                for hf in range(2):
                    ch = slice(hf * 512, (hf + 1) * 512)
                    lvl = ltpsum.tile([P, 512], F32, tag="lt", name="lvl")
                    nc.tensor.matmul(
                        lvl[0:1, :],
                        lhsT=ones_sb[0:8, :],
                        rhs=csg[:, ch],
                        start=True,
                        stop=True,
                    )
                    nc.vector.tensor_add(
                        out=corrf[:, g + 1, ch], in0=corrf[:, g, ch], in1=lvl[0:1, :]
                    )
                with nc.allow_low_precision(reason="bf16 hi+lo split of fp32 corr"):
                    nc.vector.tensor_copy(out=corrh[:, g + 1], in_=corrf[:, g + 1])
                    nc.vector.tensor_sub(
                        out=corrl[:, g + 1], in0=corrf[:, g + 1], in1=corrh[:, g + 1]
                    )

            for g in range(3):
                corr_level(g)

            def load_kt_pair(t):
                g, r = t // 8, t % 8
                kt2 = vpool.tile([P, 2, 8, 256], FP8, tag="kt", name="kt2")
                src_ = (
                    kv0_gath[r : r + 2, R0_KT : R0_KT + P]
                    if g == 0
                    else kvB_gath[g - 1, r : r + 2, R_KT : R_KT + P]
                )
                nc.scalar.dma_start(
                    out=kt2.rearrange("p a m c -> p a (m c)"),
                    in_=src_.bitcast(FP8).rearrange("a p c -> p a c"),
                )
                return kt2

            def load_v8_pair(t):
                g, r = t // 8, t % 8
                v82 = vpool.tile([P, 2, 2, D], FP8, tag="v8", name="v82")
                nc.scalar.dma_start(
                    out=v82.rearrange("p a b c -> p a (b c)"),
                    in_=kvB_gath[g - 1, r : r + 2, R_V8 : R_V8 + P]
                    .bitcast(FP8)
                    .rearrange("a p c -> p a c"),
                )
                return v82

            def get_v8(t, cache):
                if t < 8:
                    return v80_sb[:, t].rearrange("p (b c) -> p b c", b=2)
                if t % 2 == 0:
                    cache["v"] = load_v8_pair(t)
                return cache["v"][:, t % 2]

            def logits(kt_t, b, cols, n):
                # [128 keys, n] logit block vs q columns cols..cols+n
                lt = ltpsum.tile([P, 512], F32, tag="lt", name="lt")
                for p4 in range(4):
                    nc.tensor.matmul(
                        lt[:, 0:n],
                        lhsT=kt_t[:, 2 * p4 : 2 * p4 + 2, b * P : (b + 1) * P],
                        rhs=qt_sb[:, 2 * p4 : 2 * p4 + 2, cols : cols + n],
                        start=(p4 == 0),
                        stop=(p4 == 3),
                        perf_mode=DR,
                    )
                return lt

            def pvA(pt8_t, v8_t, h, sums, stop=False):
                for qc in range(2):
                    lhsT = pt8_t[:, :, qc * P : (qc + 1) * P]
                    for dh in range(2):
                        nc.tensor.matmul(
                            h[qc][:, dh, :],
                            lhsT=lhsT,
                            rhs=v8_t[:, :, dh * 512 : (dh + 1) * 512],
                            start=False,
                            stop=stop,
                            perf_mode=DR,
                        )
                    nc.tensor.matmul(
                        sums[:, qc : qc + 1],
                        lhsT=lhsT,
                        rhs=ones8_2,
                        start=False,
                        stop=stop,
                        perf_mode=DR,
                        skip_group_check=True,
                    )

            def pvB(j, lt, v_t, b, h, sums, jmax):
                # slot-0 boundary: bf16 masked PV against bf16 V
                pt = ptpool.tile([P, 256], BF16, tag="pt", name="pt")
                nc.scalar.activation(
                    out=pt, in_=lt[:, 0:256], func=AF.Exp, scale=EXP_SCALE
                )
                m_t = mpool.tile([P, 256], BF16, tag="m", name="m_t")
                nc.sync.dma_start(out=m_t, in_=mask_ext[j])
                nc.vector.tensor_mul(out=pt, in0=pt, in1=m_t)
                for qc in range(2):
                    lhsT = pt[:, qc * P : (qc + 1) * P]
                    for dh in range(2):
                        nc.tensor.matmul(
                            h[qc][:, dh, :],
                            lhsT=lhsT,
                            rhs=v_t[:, b, dh * 512 : (dh + 1) * 512],
                            start=False,
                            stop=(j == jmax),
                        )
                    nc.tensor.matmul(
                        sums[:, qc : qc + 1],
                        lhsT=lhsT,
                        rhs=ones_sb,
                        start=False,
                        stop=(j == jmax),
                        skip_group_check=True,
                    )

            def fwd_pt8(lt, off, n, dst):
                # exp(lt[off:off+n]) - 1 -> fp8 into a forward store slice
                ptf = ptpool.tile([P, 512], BF16, tag="ptf", name="ptf")
                with nc.allow_low_precision(reason="fp8 delta decomposition"):
                    nc.scalar.activation(
                        out=ptf[:, 0:n], in_=lt[:, off : off + n], func=AF.Exp,
                        scale=EXP_SCALE,
                    )
                    nc.vector.tensor_scalar_add(out=dst, in0=ptf[:, 0:n], scalar1=-1.0)

            def own_pt8(lt, off, bnd, j, pt8_t, b):
                # exp(lt[off:off+256]) [*mask] - 1 -> fp8 own delta
                ptf = ptpool.tile([P, 256], BF16, tag="ptfo", name="ptfo")
                with nc.allow_low_precision(reason="fp8 delta decomposition"):
                    nc.scalar.activation(
                        out=ptf, in_=lt[:, off : off + 256], func=AF.Exp,
                        scale=EXP_SCALE,
                    )
                    if bnd:
                        m_t = mpool.tile([P, 256], BF16, tag="m", name="m_t")
                        nc.sync.dma_start(out=m_t, in_=mask_ext[j])
                        nc.vector.tensor_mul(out=ptf, in0=ptf, in1=m_t)
                    nc.vector.tensor_scalar_add(
                        out=pt8_t[:, b, :], in0=ptf, scalar1=-1.0
                    )

            def seed(s, h, sums):
                nc.tensor.matmul(
                    sums,
                    lhsT=ones_row,
                    rhs=cnt_sb[:, 2 * s : 2 * s + 2],
                    start=True,
                    stop=False,
                    skip_group_check=True,
                )
                for qc in range(2):
                    for dh in range(2):
                        nc.tensor.matmul(
                            h[qc][:, dh, :],
                            lhsT=ones_row,
                            rhs=corrh[:, s, dh * 512 : (dh + 1) * 512],
                            start=True,
                            stop=False,
                        )
                        nc.tensor.matmul(
                            h[qc][:, dh, :],
                            lhsT=ones_row,
                            rhs=corrl[:, s, dh * 512 : (dh + 1) * 512],
                            start=False,
                            stop=False,
                        )

            def cs8_seed(s, h):
                csg8 = epool.tile([8, 2, D], BF16, tag="csg8", name=f"cs8_{s}")
                nc.scalar.dma_start(
                    out=csg8, in_=kvB_gath[s - 1, :, R_CS8 : R_CS8 + 2]
                )
                for qc in range(2):
                    for dh in range(2):
                        for hl in range(2):
                            nc.tensor.matmul(
                                h[qc][:, dh, :],
                                lhsT=ones8w,
                                rhs=csg8[:, hl, dh * 512 : (dh + 1) * 512],
                                start=False,
                                stop=False,
                            )

            def epilogue(s, h, sums):
                g_bf = []
                for qc in range(2):
                    recip = epool.tile([P, 1], F32, tag="recip", name="recip")
                    nc.vector.reciprocal(out=recip, in_=sums[:, qc : qc + 1])
                    g = gpool.tile([P, D], BF16, tag=f"g{qc}", name=f"g{qc}")
                    nc.scalar.activation(
                        out=g,
                        in_=h[qc].rearrange("p a b -> p (a b)"),
                        func=AF.Silu,
                        scale=recip,
                    )
                    g_bf.append(g)
                gt_sb = epool.tile([P, 8, 256], BF16, tag="gt", name="gt")
                for m in range(8):
                    for qc in range(2):
                        tp = ltpsum.tile([P, 512], F32, tag="lt", name="tp")
                        tpb = tp.bitcast(BF16)
                        nc.tensor.transpose(
                            tpb[:, :P],
                            g_bf[qc][:, m * P : (m + 1) * P],
                            ident_sb,
                        )
                        nc.vector.tensor_copy(
                            out=gt_sb[:, m, qc * P : (qc + 1) * P], in_=tpb[:, :P]
                        )
                for qc in range(2):
                    op = hpsum.tile([P, 2, 512], F32, tag=f"hq{qc}", name=f"o{qc}_{s}")
                    for m in range(8):
                        for dh in range(2):
                            nc.tensor.matmul(
                                op[:, dh, :],
                                lhsT=gt_sb[:, m, qc * P : (qc + 1) * P],
                                rhs=wv2_sb[:, m, dh * 512 : (dh + 1) * 512],
                                start=(m == 0),
                                stop=(m == 7),
                            )
                    oo = epool.tile([P, 2, 512], F32, tag="oo", name="oo")
                    nc.vector.tensor_copy(out=oo, in_=op)
                    nc.sync.dma_start(
                        out=o_ext[s, qc], in_=oo.rearrange("p a b -> p (a b)")
                    )

            # level-0 v8 (used by every pass's tiles 0-7 PV)
            for t in range(8):
                nc.scalar.dma_start(
                    out=v80_sb[:, t],
                    in_=kv0_gath[t, R0_V8 : R0_V8 + P].bitcast(FP8),
                )

            # ---- passes in order 1, 2, 3, 0 -----------------------------
            for s in range(1, NSLOTS):
                h = [
                    hpsum.tile([P, 2, 512], F32, tag=f"hq{qc}", name=f"h{qc}_{s}")
                    for qc in range(2)
                ]
                sums = spsum.tile([P, 2], F32, tag="sums", name="sums")
                seed(s, h, sums)
                pend = []
                cache = {}
                seeded_cs8 = False
                for t in range(8 * (s + 1)):
                    bnd = t >= 8 * s
                    if bnd and not seeded_cs8:
                        cs8_seed(s, h)
                        seeded_cs8 = True
                    v8_t = get_v8(t, cache)
                    if s >= 2 and t < 8 * s and t < 24:
                        # stored deltas from an earlier pass
                        if t < 8:
                            pt8_t = pt8s0[:, t, :, 256 * (s - 2) : 256 * (s - 1)]
                        elif t < 16:
                            pt8_t = pt8s1[:, t - 8, :, 256 * (s - 2) : 256 * (s - 1)]
                        else:
                            pt8_t = pt8s2[:, t - 16, :, 0:256]
                        if len(pend) == 2:
                            pvA(*pend.pop(0), h, sums)
                        pend.append((pt8_t, v8_t))
                        continue
                    # fresh logits: own + forwards (slot 1 on tiles 0-7 also
                    # forwards slot 0's masked bf16 P)
                    if t % 2 == 0:
                        cache["kt"] = load_kt_pair(t)
                    kt_t = cache["kt"][:, t % 2]
                    pt8_t = ptpool.tile([P, 2, 256], FP8, tag="pt8", name="pt8")
                    for b in range(2):
                        j = 2 * t + b
                        lt2 = None
                        if s == 1 and t < 8:
                            # [slot0|slot1] and [slot2|slot3] blocks
                            lt = logits(kt_t, b, 0, 512)
                            lt2 = logits(kt_t, b, 512, 512)
                        elif s < 3:
                            lt = logits(kt_t, b, s * 256, 512)
                            if s == 1:
                                lt2 = logits(kt_t, b, 768, 256)
                        else:
                            lt = logits(kt_t, b, s * 256, 256)
                        if len(pend) == 2 and b == 0:
                            pvA(*pend.pop(0), h, sums)
                        if s == 1 and t < 8:
                            # slot0 masked bf16 P forward
                            pt0 = ptpool.tile([P, 256], BF16, tag="pt0", name="pt0")
                            nc.scalar.activation(
                                out=pt0, in_=lt[:, 0:256], func=AF.Exp,
                                scale=EXP_SCALE,
                            )
                            m_t = mpool.tile([P, 256], BF16, tag="m", name="m_t")
                            nc.sync.dma_start(out=m_t, in_=mask_ext[j])
                            nc.vector.tensor_mul(
                                out=pt0_sb[:, t, b, :], in0=pt0, in1=m_t
                            )
                            own_pt8(lt, 256, False, j, pt8_t, b)
                            fwd_pt8(lt2, 0, 512, pt8s0[:, t, b, 0:512])
                        else:
                            own_pt8(lt, 0, bnd, j, pt8_t, b)
                            if s == 1:
                                fwd_pt8(lt, 256, 256, pt8s1[:, t - 8, b, 0:256])
                                fwd_pt8(lt2, 0, 256, pt8s1[:, t - 8, b, 256:512])
                            elif s == 2 and bnd:
                                fwd_pt8(lt, 256, 256, pt8s2[:, t - 16, b, 0:256])
                    pend.append((pt8_t, v8_t))
                while pend:
                    last = len(pend) == 1
                    pvA(*pend.pop(0), h, sums, stop=last)
                epilogue(s, h, sums)

            # ---- slot 0 pass (last): PV over forwarded masked bf16 P -----
            h = [
                hpsum.tile([P, 2, 512], F32, tag=f"hq{qc}", name=f"h{qc}_0")
                for qc in range(2)
            ]
            sums = spsum.tile([P, 2], F32, tag="sums", name="sums")
            seed(0, h, sums)
            for t in range(8):
                v_t = vpool.tile([P, 2, D], BF16, tag="v", name="v2")
                nc.sync.dma_start(
                    out=v_t,
                    in_=kv0_gath[t, R0_V16 : R0_V16 + 2 * P].rearrange(
                        "(b p) d -> p b d", p=P
                    ),
                )
                for b in range(2):
                    j = 2 * t + b
                    pt = pt0_sb[:, t, b, :]
                    for qc in range(2):
                        lhsT = pt[:, qc * P : (qc + 1) * P]
                        for dh in range(2):
                            nc.tensor.matmul(
                                h[qc][:, dh, :],
                                lhsT=lhsT,
                                rhs=v_t[:, b, dh * 512 : (dh + 1) * 512],
                                start=False,
                                stop=(j == 15),
                            )
                        nc.tensor.matmul(
                            sums[:, qc : qc + 1],
                            lhsT=lhsT,
                            rhs=ones_sb,
                            start=False,
                            stop=(j == 15),
                            skip_group_check=True,
                        )
            epilogue(0, h, sums)

        singles_ctx.__exit__(None, None, None)

    nc.finalize()
    return nc


_NC_CACHE = {}


def get_nc():
    if "nc" not in _NC_CACHE:
        _NC_CACHE["nc"] = build_kernel()
    return _NC_CACHE["nc"]


def build_masks():
    """Masks for the last 16 visits of each slot, selected per core by
    k = 2c + 16s - j: k>=1 all-visible, k==0 upper-left triangle, k==-1
    shifted triangle, k<=-2 fully masked (padded visit)."""
    p = np.arange(P)[:, None]
    u = np.arange(256)[None, :]
    m_ones = np.ones((P, 256), np.float32)
    m0 = (p <= u).astype(np.float32)
    m1 = (p <= u - P).astype(np.float32)
    m_zero = np.zeros((P, 256), np.float32)
    canon = np.stack([m_zero, m1, m0, m_ones]).astype(ml_dtypes.bfloat16)

    out = []
    for c in range(NCORES):
        sel = []
        for s in range(NSLOTS):
            for j in range(16 * s, 16 * (s + 1)):
                k = 2 * c + 16 * s - j
                sel.append(min(max(k, -2), 1) + 2)
        out.append(canon[np.array(sel, np.int64)])
    return out  # list of [64, 128, 256] bf16


def build_in_maps(x, wq, wk, wv1, wv2):
    bf = ml_dtypes.bfloat16
    f8 = ml_dtypes.float8_e4m3
    xT = np.ascontiguousarray(np.asarray(x, np.float32).T)
    masks = build_masks()
    w = {
        "wq": (np.asarray(wq, np.float32) * WSCALE).astype(f8),
        "wk": (np.asarray(wk, np.float32) * WSCALE).astype(f8),
        "wv1": np.asarray(wv1, np.float32).astype(bf),
        "wv2": np.asarray(wv2, np.float32).astype(bf),
    }
    in_maps = []
    for c in range(NCORES):
        xq_c = np.ascontiguousarray(
            np.concatenate(
                [
                    xT[:, 256 * (c + 8 * s) : 256 * (c + 8 * s) + 256]
                    for s in range(NSLOTS)
                ],
                axis=1,
            )
        )
        in_maps.append(
            {
                "xb": xq_c.astype(bf),
                "x8": xq_c.astype(f8),
                "masks": masks[c],
                **w,
            }
        )
    return in_maps


def assemble_out(results):
    out = np.empty((SEQ, D), np.float32)
    for c in range(NCORES):
        o = results[c]["o"]  # [4, 2, 128, 1024]
        for s in range(NSLOTS):
            r0 = 256 * (c + 8 * s)
            out[r0 : r0 + P, :] = o[s, 0]
            out[r0 + P : r0 + 256, :] = o[s, 1]
    return out


def kernel(x, wq, wk, wv1, wv2):
    in_maps = build_in_maps(x, wq, wk, wv1, wv2)
    nc = get_nc()
    res = run_bass_kernel_spmd(nc, in_maps, list(range(NCORES)))
    return assemble_out(res.results)


# revision 32
# speedup vs baseline: 1.0784x; 1.0784x over previous
"""Trainium2 Bass kernel for nn_MemoryAttention (causal single-head attention
with SiLU-gated output projection), sequence-parallel across 8 NeuronCores.

v5 strategy (per core c):
  - q rows owned: 4 slots of 256 rows: tile t = c + 8*s (strided assignment
    balances causal work; every core runs an identical instruction stream).
  - fp8e4 (DoubleRow) for Q/K projections, QK^T logits, and all of slots
    1-3's PV via the delta decomposition P = 1 + delta with multiplicative
    masks folded in before the -1 (masked keys give delta = -1 exactly,
    cancelled by hi+lo bf16 colsum(V8) seeds computed on the producer).
  - Slot 0 (fully boundary, small rowsums) keeps the bf16 masked path over
    bf16 V shipped only for level 0.
  - Logits are computed ONCE per key-tile for every slot that sees it, at
    free dim 512 (2 q-slots per matmul), during the EARLIEST slot's pass;
    later slots' deltas are forwarded through fp8 SBUF stores (~24KB/part).
  - Two collectives only: G0 (slot-0 payload: kt|v8|v16|colsums) posted
    ~20us in, and G123 (slots 1-3: kt|v8|colsum8 hi+lo) posted right after
    the last V projection.  Collectives here cost ~35us nearly independent
    of size, so fewer is faster.
  - Slot epilogue: H / rowsums, SiLU, DMA-transpose (XBAR) of G, out proj.
"""

import numpy as np
import ml_dtypes

import concourse.bass as bass
import concourse.tile as tile
from concourse import bacc, mybir
from concourse.bass_utils import run_bass_kernel_spmd
from concourse.masks import make_identity

P = 128
D = 1024
SEQ = 8192
NCORES = 8
NSLOTS = 4
WSCALE = 64.0
EXP_SCALE = 0.03125 / (WSCALE * WSCALE)

# slot-0 payload rows (units of [., 1024] bf16): kt fp8 | v8 fp8 | v bf16 | cs
R0_KT = 0
R0_V8 = P
R0_V16 = 2 * P
R0_CS = 4 * P
R0_TOT = 4 * P + 3
# slots 1-3 payload rows: kt fp8 | v8 fp8 | colsum(v8) bf16 hi+lo
R_KT = 0
R_V8 = P
R_CS8 = 2 * P
R_TOT = 2 * P + 2

F32 = mybir.dt.float32
BF16 = mybir.dt.bfloat16
FP8 = mybir.dt.float8e4
AF = mybir.ActivationFunctionType
DR = mybir.MatmulPerfMode.DoubleRow
AX = mybir.AxisListType
ALU = mybir.AluOpType


def build_kernel():
    nc = bacc.Bacc(None, target_bir_lowering=False, num_devices=NCORES)

    xb_ext = nc.declare_dram_parameter("xb", [D, D], BF16, isOutput=False)
    x8_ext = nc.declare_dram_parameter("x8", [D, D], FP8, isOutput=False)
    wq_ext = nc.declare_dram_parameter("wq", [D, D], FP8, isOutput=False)
    wk_ext = nc.declare_dram_parameter("wk", [D, D], FP8, isOutput=False)
    wv1_ext = nc.declare_dram_parameter("wv1", [D, D], BF16, isOutput=False)
    wv18_ext = nc.declare_dram_parameter("wv18", [D, D], FP8, isOutput=False)
    wv2_ext = nc.declare_dram_parameter("wv2", [D, D], BF16, isOutput=False)
    mask_ext = nc.declare_dram_parameter("masks", [64, P, 256], BF16, isOutput=False)
    o_ext = nc.declare_dram_parameter("o", [NSLOTS, 2, P, D], F32, isOutput=True)

    kv0_local = nc.dram_tensor("kv0_local", [R0_TOT, D], BF16)
    kv0_gath = nc.dram_tensor("kv0_gath", [NCORES, R0_TOT, D], BF16, addr_space="Shared")
    kv_local = nc.dram_tensor("kv_local", [3, R_TOT, D], BF16)
    kvB_gath = nc.dram_tensor(
        "kvB_gath", [3, NCORES, R_TOT, D], BF16, addr_space="Shared"
    )

    def wload(nc, pool, ext, tag, dt):
        t = pool.tile([P, 8, D], dt, tag=tag, name=tag)
        nc.sync.dma_start(out=t, in_=ext[:].rearrange("(sub p) s -> p sub s", p=P))
        return t

    with tile.TileContext(nc) as tc:
        singles_ctx = tc.tile_pool(name="singles", bufs=1)
        singles = singles_ctx.__enter__()

        ones_sb = singles.tile([P, 1], BF16)
        nc.vector.memset(ones_sb, 1.0)
        ones_row = singles.tile([1, P], BF16)
        nc.vector.memset(ones_row, 1.0)
        ones8_2 = singles.tile([P, 2, 1], FP8)
        nc.vector.memset(ones8_2, 1.0)
        ones8_1 = singles.tile([P, 1], FP8)
        nc.vector.memset(ones8_1, 1.0)
        ones8w = singles.tile([8, P], BF16)
        nc.vector.memset(ones8w, 1.0)
        cnt_sb = singles.tile([1, 2 * NSLOTS], BF16)
        for s in range(NSLOTS):
            cval = 0.0 if s == 0 else float(2048 * (s + 1))
            nc.vector.memset(cnt_sb[:, 2 * s : 2 * s + 2], cval)
        ident_sb = singles.tile([P, P], BF16)
        make_identity(nc, ident_sb)
        qt_sb = singles.tile([P, 8, D], FP8)

        with (
            tc.tile_pool(name="projw", bufs=1) as projw,
            tc.tile_pool(name="projout", bufs=4) as projout,
            tc.tile_pool(name="ppsum", bufs=1, space="PSUM") as ppsum,
            tc.tile_pool(name="vpsum", bufs=4, space="PSUM") as vpsum,
            tc.tile_pool(name="cpsum", bufs=1, space="PSUM") as cpsum,
        ):
            def pairload(pool, ext, tag, dt):
                t = pool.tile([P, 8, D], dt, tag=tag, name=tag)
                v = ext[:].rearrange("(sub p) s -> p sub s", p=P)
                for p4 in range(4):
                    nc.sync.dma_start(
                        out=t[:, 2 * p4 : 2 * p4 + 2, :], in_=v[:, 2 * p4 : 2 * p4 + 2, :]
                    )
                return t

            x8_v = x8_ext[:].rearrange("(sub p) s -> p sub s", p=P)
            wk_v = wk_ext[:].rearrange("(sub p) s -> p sub s", p=P)
            x8_sb = projw.tile([P, 8, D], FP8, tag="x8", name="x8")
            wk8 = projw.tile([P, 8, D], FP8, tag="wk", name="wk")
            for p4 in range(4):
                pr = slice(2 * p4, 2 * p4 + 2)
                nc.sync.dma_start(out=wk8[:, pr, :], in_=wk_v[:, pr, :])
                nc.sync.dma_start(out=x8_sb[:, pr, :], in_=x8_v[:, pr, :])
            xb_sb = pairload(projw, xb_ext, "xb", BF16)
            wv1_sb = pairload(projw, wv1_ext, "wv1", BF16)
            wv18_sb = pairload(projw, wv18_ext, "wv18", FP8)
            wq8 = pairload(projw, wq_ext, "wq", FP8)

            def kqt_proj(w8, cols, out_cb):
                for m in range(8):
                    acc = ppsum.tile([P, 512], F32, tag=f"proj{m % 2}", name=f"kq{m}")
                    for p4 in range(4):
                        nc.tensor.matmul(
                            acc,
                            lhsT=w8[:, 2 * p4 : 2 * p4 + 2, m * P : (m + 1) * P],
                            rhs=x8_sb[:, 2 * p4 : 2 * p4 + 2, cols],
                            start=(p4 == 0),
                            stop=(p4 == 3),
                            perf_mode=DR,
                        )
                    out_cb(m, acc)

            def gather(in_ap, out_ap):
                nc.gpsimd.collective_compute(
                    "AllGather",
                    mybir.AluOpType.bypass,
                    replica_groups=[list(range(NCORES))],
                    ins=[in_ap],
                    outs=[out_ap],
                )

            def kproj_pair(p01):
                cols = slice(p01 * 512, (p01 + 1) * 512)
                kt_out = projout.tile([P, 8, 512], FP8, tag="kt_out", name="kto")
                kqt_proj(
                    wk8,
                    cols,
                    lambda m, acc: nc.vector.tensor_copy(out=kt_out[:, m, :], in_=acc),
                )
                for gg in range(2):
                    g = 2 * p01 + gg
                    dst = (
                        kv0_local[R0_KT : R0_KT + P]
                        if g == 0
                        else kv_local[g - 1, R_KT : R_KT + P]
                    )
                    nc.sync.dma_start(
                        out=dst.bitcast(FP8).rearrange("p (m c) -> p m c", m=8),
                        in_=kt_out[:, :, gg * 256 : (gg + 1) * 256],
                    )

            def vproj(g):
                v8_out = projout.tile([P, 2, D], FP8, tag="v8_out", name="v8o")
                for blk in range(2):
                    accs = [
                        vpsum.tile([P, 512], F32, tag="vproj", name=f"vp{h2}")
                        for h2 in range(2)
                    ]
                    bc = slice(g * 256 + blk * P, g * 256 + (blk + 1) * P)
                    if g == 0:
                        # exact bf16 path (feeds v16 and slot-0 precision)
                        for sub in range(8):
                            for h2 in range(2):
                                nc.tensor.matmul(
                                    accs[h2],
                                    lhsT=xb_sb[:, sub, bc],
                                    rhs=wv1_sb[:, sub, h2 * 512 : (h2 + 1) * 512],
                                    start=(sub == 0),
                                    stop=(sub == 7),
                                )
                    else:
                        # fp8 DoubleRow (x64-scaled wv1; scale folded out in
                        # the fp8 copy below)
                        for p4 in range(4):
                            for h2 in range(2):
                                nc.tensor.matmul(
                                    accs[h2],
                                    lhsT=x8_sb[:, 2 * p4 : 2 * p4 + 2, bc],
                                    rhs=wv18_sb[
                                        :, 2 * p4 : 2 * p4 + 2,
                                        h2 * 512 : (h2 + 1) * 512,
                                    ],
                                    start=(p4 == 0),
                                    stop=(p4 == 3),
                                    perf_mode=DR,
                                )
                    with nc.allow_low_precision(reason="fp8 V feeds the delta path"):
                        for h2 in range(2):
                            nc.scalar.activation(
                                out=v8_out[:, blk, h2 * 512 : (h2 + 1) * 512],
                                in_=accs[h2],
                                func=AF.Copy,
                                scale=(1.0 if g == 0 else 1.0 / WSCALE),
                            )
                    if g == 0:
                        v_out = projout.tile([P, D], BF16, tag="v_out", name="vo")
                        for h2 in range(2):
                            nc.vector.tensor_copy(
                                out=v_out[:, h2 * 512 : (h2 + 1) * 512], in_=accs[h2]
                            )
                        nc.sync.dma_start(
                            out=kv0_local[R0_V16 + blk * P : R0_V16 + (blk + 1) * P],
                            in_=v_out,
                        )
                v8_dst = (
                    kv0_local[R0_V8 : R0_V8 + P]
                    if g == 0
                    else kv_local[g - 1, R_V8 : R_V8 + P]
                )
                nc.sync.dma_start(
                    out=v8_dst.bitcast(FP8), in_=v8_out.rearrange("p b d -> p (b d)")
                )
                if g > 0:
                    # colsum of the shipped v8 (fp32 psum), hi+lo bf16 split
                    # so it cancels the masked-key deltas to ~1e-4
                    cs = cpsum.tile([1, D], F32, tag="cs", name="cs8")
                    for dh in range(2):
                        for b in range(2):
                            nc.tensor.matmul(
                                cs[:, dh * 512 : (dh + 1) * 512],
                                lhsT=ones8_1,
                                rhs=v8_out[:, b, dh * 512 : (dh + 1) * 512],
                                start=(b == 0),
                                stop=(b == 1),
                            )
                    csh = projout.tile([1, D], BF16, tag="cs8h", name="cs8h")
                    csl = projout.tile([1, D], BF16, tag="cs8l", name="cs8l")
                    with nc.allow_low_precision(reason="bf16 hi+lo split of fp32 cs8"):
                        nc.vector.tensor_copy(out=csh, in_=cs)
                        nc.vector.tensor_sub(out=csl, in0=cs, in1=csh)
                    nc.sync.dma_start(out=kv_local[g - 1, R_CS8 : R_CS8 + 1], in_=csh)
                    nc.sync.dma_start(
                        out=kv_local[g - 1, R_CS8 + 1 : R_CS8 + 2], in_=csl
                    )

            def ship_exact_colsums():
                xsumL = projw.tile([P, 8, 3], BF16, tag="xsumL", name="xsumL")
                with nc.allow_low_precision(reason="bf16 colsum feeds hi+lo split"):
                    for lvl in range(3):
                        nc.vector.tensor_reduce(
                            out=xsumL[:, :, lvl : lvl + 1],
                            in_=xb_sb[:, :, lvl * 256 : (lvl + 1) * 256],
                            axis=AX.X,
                            op=ALU.add,
                        )
                for lvl in range(3):
                    cs = cpsum.tile([1, D], F32, tag="cs", name="cs")
                    for sub in range(8):
                        for dh in range(2):
                            nc.tensor.matmul(
                                cs[:, dh * 512 : (dh + 1) * 512],
                                lhsT=xsumL[:, sub, lvl : lvl + 1],
                                rhs=wv1_sb[:, sub, dh * 512 : (dh + 1) * 512],
                                start=(sub == 0),
                                stop=(sub == 7),
                            )
                    csb = projout.tile([1, D], BF16, tag="csb", name="csb")
                    nc.vector.tensor_copy(out=csb, in_=cs)
                    nc.sync.dma_start(
                        out=kv0_local[R0_CS + lvl : R0_CS + lvl + 1], in_=csb
                    )

            def qproj_pair(p01):
                cols = slice(p01 * 512, (p01 + 1) * 512)
                kqt_proj(
                    wq8,
                    cols,
                    lambda m, acc: nc.vector.tensor_copy(
                        out=qt_sb[:, m, p01 * 512 : (p01 + 1) * 512], in_=acc
                    ),
                )

            kproj_pair(0)
            ship_exact_colsums()
            vproj(0)
            gather(kv0_local[:], kv0_gath[:])
            qproj_pair(0)
            kproj_pair(1)
            vproj(1)
            vproj(2)
            vproj(3)
            gather(kv_local[0], kvB_gath[0])
            gather(kv_local[1], kvB_gath[1])
            gather(kv_local[2], kvB_gath[2])
            qproj_pair(1)

        # ---- attention ----------------------------------------------------
        with (
            tc.tile_pool(name="asingles", bufs=1) as asingles,
            tc.tile_pool(name="vpool", bufs=4) as vpool,
            tc.tile_pool(name="mpool", bufs=3) as mpool,
            tc.tile_pool(name="ptpool", bufs=3) as ptpool,
            tc.tile_pool(name="epool", bufs=2) as epool,
            tc.tile_pool(name="gpool", bufs=1) as gpool,
            tc.tile_pool(name="ltpsum", bufs=3, space="PSUM") as ltpsum,
            tc.tile_pool(name="hpsum", bufs=1, space="PSUM") as hpsum,
            tc.tile_pool(name="spsum", bufs=1, space="PSUM") as spsum,
        ):
            wv2_sb = wload(nc, asingles, wv2_ext, "wv2", BF16)
            v80_sb = asingles.tile([P, 8, 2048], FP8)
            # forwarded fp8 deltas: tiles 0-7 for slots 1,2,3; 8-15 for 2,3;
            # 16-23 for 3.  Written by the earliest pass, read by later ones.
            pt8s0 = asingles.tile([P, 8, 2, 768], FP8)
            pt8s1 = asingles.tile([P, 8, 2, 512], FP8)
            pt8s2 = asingles.tile([P, 8, 2, 256], FP8)

            corrf = asingles.tile([1, NSLOTS, D], F32)
            corrh = asingles.tile([1, NSLOTS, D], BF16)
            corrl = asingles.tile([1, NSLOTS, D], BF16)
            nc.vector.memset(corrf[:, 0], 0.0)
            nc.vector.memset(corrh[:, 0], 0.0)
            nc.vector.memset(corrl[:, 0], 0.0)

            def corr_level(g):
                csg = epool.tile([8, D], BF16, tag="csg", name=f"csg{g}")
                nc.sync.dma_start(out=csg, in_=kv0_gath[:, R0_CS + g])
                for hf in range(2):
                    ch = slice(hf * 512, (hf + 1) * 512)
                    lvl = ltpsum.tile([P, 512], F32, tag="lt", name="lvl")
                    nc.tensor.matmul(
                        lvl[0:1, :],
                        lhsT=ones_sb[0:8, :],
                        rhs=csg[:, ch],
                        start=True,
                        stop=True,
                    )
                    nc.vector.tensor_add(
                        out=corrf[:, g + 1, ch], in0=corrf[:, g, ch], in1=lvl[0:1, :]
                    )
                with nc.allow_low_precision(reason="bf16 hi+lo split of fp32 corr"):
                    nc.vector.tensor_copy(out=corrh[:, g + 1], in_=corrf[:, g + 1])
                    nc.vector.tensor_sub(
                        out=corrl[:, g + 1], in0=corrf[:, g + 1], in1=corrh[:, g + 1]
                    )

            for g in range(3):
                corr_level(g)

            def load_kt_pair(t):
                g, r = t // 8, t % 8
                kt2 = vpool.tile([P, 2, 8, 256], FP8, tag="kt", name="kt2")
                nc.sync.dma_start(
                    out=kt2.rearrange("p a m c -> p a (m c)"),
                    in_=kvB_gath[g - 1, r : r + 2, R_KT : R_KT + P]
                    .bitcast(FP8)
                    .rearrange("a p c -> p a c"),
                )
                return kt2

            def load_v8_pair(t):
                g, r = t // 8, t % 8
                v82 = vpool.tile([P, 2, 2, D], FP8, tag="v8", name="v82")
                nc.sync.dma_start(
                    out=v82.rearrange("p a b c -> p a (b c)"),
                    in_=kvB_gath[g - 1, r : r + 2, R_V8 : R_V8 + P]
                    .bitcast(FP8)
                    .rearrange("a p c -> p a c"),
                )
                return v82

            def get_v8(t, cache):
                if t < 8:
                    return v80_sb[:, t].rearrange("p (b c) -> p b c", b=2)
                if t % 2 == 0:
                    cache["v"] = load_v8_pair(t)
                return cache["v"][:, t % 2]

            def logits(kt_t, b, cols, n):
                # [128 keys, n] logit block vs q columns cols..cols+n
                lt = ltpsum.tile([P, 512], F32, tag="lt", name="lt")
                for p4 in range(4):
                    nc.tensor.matmul(
                        lt[:, 0:n],
                        lhsT=kt_t[:, 2 * p4 : 2 * p4 + 2, b * P : (b + 1) * P],
                        rhs=qt_sb[:, 2 * p4 : 2 * p4 + 2, cols : cols + n],
                        start=(p4 == 0),
                        stop=(p4 == 3),
                        perf_mode=DR,
                    )
                return lt

            def pvA(pt8_t, v8_t, h, sums, stop=False):
                for qc in range(2):
                    lhsT = pt8_t[:, :, qc * P : (qc + 1) * P]
                    for dh in range(2):
                        nc.tensor.matmul(
                            h[qc][:, dh, :],
                            lhsT=lhsT,
                            rhs=v8_t[:, :, dh * 512 : (dh + 1) * 512],
                            start=False,
                            stop=stop,
                            perf_mode=DR,
                        )
                    nc.tensor.matmul(
                        sums[:, qc : qc + 1],
                        lhsT=lhsT,
                        rhs=ones8_2,
                        start=False,
                        stop=stop,
                        perf_mode=DR,
                        skip_group_check=True,
                    )

            def pvB(j, lt, v_t, b, h, sums, jmax):
                # slot-0 boundary: bf16 masked PV against bf16 V
                pt = ptpool.tile([P, 256], BF16, tag="pt", name="pt")
                nc.scalar.activation(
                    out=pt, in_=lt[:, 0:256], func=AF.Exp, scale=EXP_SCALE
                )
                m_t = mpool.tile([P, 256], BF16, tag="m", name="m_t")
                nc.sync.dma_start(out=m_t, in_=mask_ext[j])
                nc.vector.tensor_mul(out=pt, in0=pt, in1=m_t)
                for qc in range(2):
                    lhsT = pt[:, qc * P : (qc + 1) * P]
                    for dh in range(2):
                        nc.tensor.matmul(
                            h[qc][:, dh, :],
                            lhsT=lhsT,
                            rhs=v_t[:, b, dh * 512 : (dh + 1) * 512],
                            start=False,
                            stop=(j == jmax),
                        )
                    nc.tensor.matmul(
                        sums[:, qc : qc + 1],
                        lhsT=lhsT,
                        rhs=ones_sb,
                        start=False,
                        stop=(j == jmax),
                        skip_group_check=True,
                    )

            def fwd_pt8(lt, off, n, dst):
                # exp(lt[off:off+n]) - 1 -> fp8 into a forward store slice
                ptf = ptpool.tile([P, 512], BF16, tag="ptf", name="ptf")
                with nc.allow_low_precision(reason="fp8 delta decomposition"):
                    nc.scalar.activation(
                        out=ptf[:, 0:n], in_=lt[:, off : off + n], func=AF.Exp,
                        scale=EXP_SCALE,
                    )
                    nc.vector.tensor_scalar_add(out=dst, in0=ptf[:, 0:n], scalar1=-1.0)

            def own_pt8(lt, off, bnd, j, pt8_t, b):
                # exp(lt[off:off+256]) [*mask] - 1 -> fp8 own delta
                ptf = ptpool.tile([P, 256], BF16, tag="ptfo", name="ptfo")
                with nc.allow_low_precision(reason="fp8 delta decomposition"):
                    nc.scalar.activation(
                        out=ptf, in_=lt[:, off : off + 256], func=AF.Exp,
                        scale=EXP_SCALE,
                    )
                    if bnd:
                        m_t = mpool.tile([P, 256], BF16, tag="m", name="m_t")
                        nc.sync.dma_start(out=m_t, in_=mask_ext[j])
                        nc.vector.tensor_mul(out=ptf, in0=ptf, in1=m_t)
                    nc.vector.tensor_scalar_add(
                        out=pt8_t[:, b, :], in0=ptf, scalar1=-1.0
                    )

            def seed(s, h, sums):
                nc.tensor.matmul(
                    sums,
                    lhsT=ones_row,
                    rhs=cnt_sb[:, 2 * s : 2 * s + 2],
                    start=True,
                    stop=False,
                    skip_group_check=True,
                )
                for qc in range(2):
                    for dh in range(2):
                        nc.tensor.matmul(
                            h[qc][:, dh, :],
                            lhsT=ones_row,
                            rhs=corrh[:, s, dh * 512 : (dh + 1) * 512],
                            start=True,
                            stop=False,
                        )
                        nc.tensor.matmul(
                            h[qc][:, dh, :],
                            lhsT=ones_row,
                            rhs=corrl[:, s, dh * 512 : (dh + 1) * 512],
                            start=False,
                            stop=False,
                        )

            def cs8_seed(s, h):
                csg8 = epool.tile([8, 2, D], BF16, tag="csg8", name=f"cs8_{s}")
                nc.sync.dma_start(
                    out=csg8, in_=kvB_gath[s - 1, :, R_CS8 : R_CS8 + 2]
                )
                for qc in range(2):
                    for dh in range(2):
                        for hl in range(2):
                            nc.tensor.matmul(
                                h[qc][:, dh, :],
                                lhsT=ones8w,
                                rhs=csg8[:, hl, dh * 512 : (dh + 1) * 512],
                                start=False,
                                stop=False,
                            )

            def epilogue(s, h, sums):
                g_bf = []
                for qc in range(2):
                    recip = epool.tile([P, 1], F32, tag="recip", name="recip")
                    nc.vector.reciprocal(out=recip, in_=sums[:, qc : qc + 1])
                    g = gpool.tile([P, D], BF16, tag=f"g{qc}", name=f"g{qc}")
                    nc.scalar.activation(
                        out=g,
                        in_=h[qc].rearrange("p a b -> p (a b)"),
                        func=AF.Silu,
                        scale=recip,
                    )
                    g_bf.append(g)
                gt_sb = epool.tile([P, 8, 256], BF16, tag="gt", name="gt")
                for m in range(8):
                    for qc in range(2):
                        tp = ltpsum.tile([P, 512], F32, tag="lt", name="tp")
                        tpb = tp.bitcast(BF16)
                        nc.tensor.transpose(
                            tpb[:, :P],
                            g_bf[qc][:, m * P : (m + 1) * P],
                            ident_sb,
                        )
                        nc.vector.tensor_copy(
                            out=gt_sb[:, m, qc * P : (qc + 1) * P], in_=tpb[:, :P]
                        )
                for qc in range(2):
                    op = hpsum.tile([P, 2, 512], F32, tag=f"hq{qc}", name=f"o{qc}_{s}")
                    for m in range(8):
                        for dh in range(2):
                            nc.tensor.matmul(
                                op[:, dh, :],
                                lhsT=gt_sb[:, m, qc * P : (qc + 1) * P],
                                rhs=wv2_sb[:, m, dh * 512 : (dh + 1) * 512],
                                start=(m == 0),
                                stop=(m == 7),
                            )
                    oo = epool.tile([P, 2, 512], F32, tag="oo", name="oo")
                    nc.vector.tensor_copy(out=oo, in_=op)
                    nc.sync.dma_start(
                        out=o_ext[s, qc], in_=oo.rearrange("p a b -> p (a b)")
                    )

            # ---- slot 0 pass: bf16 own path + forward logits for 1,2,3 ---
            h = [
                hpsum.tile([P, 2, 512], F32, tag=f"hq{qc}", name=f"h{qc}_0")
                for qc in range(2)
            ]
            sums = spsum.tile([P, 2], F32, tag="sums", name="sums")
            seed(0, h, sums)
            pendB = None
            for t in range(8):
                v_t = vpool.tile([P, 2, D], BF16, tag="v", name="v2")
                nc.sync.dma_start(
                    out=v_t,
                    in_=kv0_gath[t, R0_V16 : R0_V16 + 2 * P].rearrange(
                        "(b p) d -> p b d", p=P
                    ),
                )
                kt_t = vpool.tile([P, 8, 256], FP8, tag="kt0", name="kt0")
                nc.sync.dma_start(
                    out=kt_t.rearrange("p m c -> p (m c)"),
                    in_=kv0_gath[t, R0_KT : R0_KT + P].bitcast(FP8),
                )
                for b in range(2):
                    j = 2 * t + b
                    ltA = logits(kt_t, b, 0, 512)     # slot0 own | slot1
                    ltB = logits(kt_t, b, 512, 512)   # slots 2,3
                    fwd_pt8(ltA, 256, 256, pt8s0[:, t, b, 0:256])
                    fwd_pt8(ltB, 0, 512, pt8s0[:, t, b, 256:768])
                    if pendB is not None:
                        pvB(*pendB, h, sums, 15)
                    pendB = (j, ltA, v_t, b)
            pvB(*pendB, h, sums, 15)
            epilogue(0, h, sums)

            # level-0 v8 loads (used by slots 1-3 interior PV), emitted
            # after slot 0's pass so its v16/kt loads win the queue order
            for t in range(8):
                nc.sync.dma_start(
                    out=v80_sb[:, t],
                    in_=kv0_gath[t, R0_V8 : R0_V8 + P].bitcast(FP8),
                )

            # ---- slots 1-3 passes: fp8 delta path ------------------------
            for s in range(1, NSLOTS):
                h = [
                    hpsum.tile([P, 2, 512], F32, tag=f"hq{qc}", name=f"h{qc}_{s}")
                    for qc in range(2)
                ]
                sums = spsum.tile([P, 2], F32, tag="sums", name="sums")
                seed(s, h, sums)
                pend = []
                cache = {}
                seeded_cs8 = False
                for t in range(8 * (s + 1)):
                    bnd = t >= 8 * s
                    if bnd and not seeded_cs8:
                        cs8_seed(s, h)
                        seeded_cs8 = True
                    v8_t = get_v8(t, cache)
                    if t < 8 * s:
                        # stored deltas from an earlier pass
                        if t < 8:
                            pt8_t = pt8s0[:, t, :, 256 * (s - 1) : 256 * s]
                        elif t < 16:
                            pt8_t = pt8s1[:, t - 8, :, 256 * (s - 2) : 256 * (s - 1)]
                        else:
                            pt8_t = pt8s2[:, t - 16, :, 0:256]
                        if len(pend) == 2:
                            pvA(*pend.pop(0), h, sums)
                        pend.append((pt8_t, v8_t))
                        continue
                    # boundary region: fresh logits (own + forward to later
                    # slots), masked own deltas
                    if t % 2 == 0:
                        cache["kt"] = load_kt_pair(t)
                    kt_t = cache["kt"][:, t % 2]
                    nfwd = 3 - s  # number of later slots forwarded
                    pt8_t = ptpool.tile([P, 2, 256], FP8, tag="pt8", name="pt8")
                    for b in range(2):
                        j = 2 * t + b
                        lt2 = None
                        if nfwd > 0:
                            # own + first forwarded slot in one N=512 block
                            lt = logits(kt_t, b, s * 256, 512)
                            if nfwd == 2:
                                lt2 = logits(kt_t, b, (s + 2) * 256, 256)
                        else:
                            lt = logits(kt_t, b, s * 256, 256)
                        if len(pend) == 2 and b == 0:
                            pvA(*pend.pop(0), h, sums)
                        own_pt8(lt, 0, True, j, pt8_t, b)
                        if s == 1:
                            fwd_pt8(lt, 256, 256, pt8s1[:, t - 8, b, 0:256])
                            fwd_pt8(lt2, 0, 256, pt8s1[:, t - 8, b, 256:512])
                        elif s == 2:
                            fwd_pt8(lt, 256, 256, pt8s2[:, t - 16, b, 0:256])
                    pend.append((pt8_t, v8_t))
                while pend:
                    last = len(pend) == 1
                    pvA(*pend.pop(0), h, sums, stop=last)
                epilogue(s, h, sums)

        singles_ctx.__exit__(None, None, None)

    nc.finalize()
    return nc


_NC_CACHE = {}


def get_nc():
    if "nc" not in _NC_CACHE:
        _NC_CACHE["nc"] = build_kernel()
    return _NC_CACHE["nc"]


def build_masks():
    """Masks for the last 16 visits of each slot, selected per core by
    k = 2c + 16s - j: k>=1 all-visible, k==0 upper-left triangle, k==-1
    shifted triangle, k<=-2 fully masked (padded visit)."""
    p = np.arange(P)[:, None]
    u = np.arange(256)[None, :]
    m_ones = np.ones((P, 256), np.float32)
    m0 = (p <= u).astype(np.float32)
    m1 = (p <= u - P).astype(np.float32)
    m_zero = np.zeros((P, 256), np.float32)
    canon = np.stack([m_zero, m1, m0, m_ones]).astype(ml_dtypes.bfloat16)

    out = []
    for c in range(NCORES):
        sel = []
        for s in range(NSLOTS):
            for j in range(16 * s, 16 * (s + 1)):
                k = 2 * c + 16 * s - j
                sel.append(min(max(k, -2), 1) + 2)
        out.append(canon[np.array(sel, np.int64)])
    return out  # list of [64, 128, 256] bf16


def build_in_maps(x, wq, wk, wv1, wv2):
    bf = ml_dtypes.bfloat16
    f8 = ml_dtypes.float8_e4m3
    xT = np.ascontiguousarray(np.asarray(x, np.float32).T)
    masks = build_masks()
    w = {
        "wq": (np.asarray(wq, np.float32) * WSCALE).astype(f8),
        "wk": (np.asarray(wk, np.float32) * WSCALE).astype(f8),
        "wv1": np.asarray(wv1, np.float32).astype(bf),
        "wv18": (np.asarray(wv1, np.float32) * WSCALE).astype(f8),
        "wv2": np.asarray(wv2, np.float32).astype(bf),
    }
    in_maps = []
    for c in range(NCORES):
        xq_c = np.ascontiguousarray(
            np.concatenate(
                [
                    xT[:, 256 * (c + 8 * s) : 256 * (c + 8 * s) + 256]
                    for s in range(NSLOTS)
                ],
                axis=1,
            )
        )
        in_maps.append(
            {
                "xb": xq_c.astype(bf),
                "x8": xq_c.astype(f8),
                "masks": masks[c],
                **w,
            }
        )
    return in_maps


def assemble_out(results):
    out = np.empty((SEQ, D), np.float32)
    for c in range(NCORES):
        o = results[c]["o"]  # [4, 2, 128, 1024]
        for s in range(NSLOTS):
            r0 = 256 * (c + 8 * s)
            out[r0 : r0 + P, :] = o[s, 0]
            out[r0 + P : r0 + 256, :] = o[s, 1]
    return out


def kernel(x, wq, wk, wv1, wv2):
    in_maps = build_in_maps(x, wq, wk, wv1, wv2)
    nc = get_nc()
    res = run_bass_kernel_spmd(nc, in_maps, list(range(NCORES)))
    return assemble_out(res.results)
